# revision 11
# baseline (speedup 1.0000x reference)
"""Trainium2 Bass kernel for CNN+GraphSAGE (nn_CNNadgnn_cnn1ch).

Sharding: data-parallel over the 64 images for the CNN (8 imgs/core); GNN
nodes sharded by owning image-group (batch//8 == core), so the CNN->graph
pixel gather is core-local.  Edges partitioned by dst shard; per-layer node
features exchanged with an ncfw AllGather; small weights replicated.

Aggregation: nodes relabeled (per core, sorted by in-degree); for each
128-node tile, slot i gathers the i-th neighbor's row for all 128 nodes via
one indirect DMA; DVE reduce sums slot groups; ACT applies 1/deg as a
per-partition scale; PE transposes produce the feature-major operand for the
dense transforms.  All matmuls in bf16, accumulation in fp32.
"""

import numpy as np
import ml_dtypes

import bass_rust
import concourse.bass as bass
import concourse.mybir as mybir
import concourse.tile as tile
import concourse.tile_utils as tile_utils
from concourse.masks import make_identity
from concourse.vector_clock import ScopedClock

BF16 = mybir.dt.bfloat16
F32 = mybir.dt.float32
I32 = mybir.dt.int32

NC = 8          # cores
IPC = 8         # images per core
HW = 128
PIX = HW * HW   # 16384
PW = HW + 2     # padded row width
PPIX = PW * PW
KG = 16         # gather slots per DVE reduce group
DIMS = [8, 128, 128, 128, 1]

tile_utils.max_sbuf_usage = 206 * 1024

# ---------------------------------------------------------------------------
# This container's walrus supports only 1 sync-wait slot per ISA instruction;
# split extra Tile-assigned waits onto same-engine NOPs placed just before.
MAX_WAITS = 1


def _split_waits(nc, insts):
    out = []
    for inst in insts:
        si = inst.sync_info
        waits = list(si.on_wait) if si is not None and si.on_wait else []
        if len(waits) > MAX_WAITS and not isinstance(
            inst, (tile.TileBranchInst, tile.BassTileLoopBlock)
        ):
            extra, keep = waits[:-MAX_WAITS], waits[-MAX_WAITS:]
            for i in range(0, len(extra), MAX_WAITS):
                nop = bass_rust.InstNoOp(name=f"I-{nc.next_id()}", ins=[], outs=[])
                nop.engine = inst.engine
                nop.sync_info = bass_rust.SyncInfo(
                    on_wait=extra[i:i + MAX_WAITS], on_update=[]
                )
                out.append(nop)
            inst.sync_info = bass_rust.SyncInfo(
                on_wait=keep, on_update=list(si.on_update or [])
            )
        out.append(inst)
    return out


_orig_lower = tile.TileContext._lower_ordered_insts


def _lower_ordered_insts(self, ordered):
    for bb_name in list(ordered.keys()):
        ordered[bb_name] = _split_waits(self.nc, ordered[bb_name])
    return _orig_lower(self, ordered)


def _drain_and_barrier(self, tick_clock, wait_clock):
    nc = self.nc
    drain_inst = nc.sync.drain()
    wait_clock.add_sem_waits(
        drain_inst.ins, ScopedClock({None: tick_clock.global_clock})
    )
    si = drain_inst.ins.sync_info
    waits = list(si.on_wait) if si is not None and si.on_wait else []
    if len(waits) > MAX_WAITS:
        drain_inst.ins.sync_info = bass_rust.SyncInfo(
            on_wait=waits[:MAX_WAITS], on_update=list(si.on_update or [])
        )
        extra = waits[MAX_WAITS:]
        for i in range(0, len(extra), MAX_WAITS):
            nop = nc.sync.nop(nofuse=True)
            nop.ins.sync_info = bass_rust.SyncInfo(
                on_wait=extra[i:i + MAX_WAITS], on_update=[]
            )
    nc.all_engine_barrier()
    assert self.sems is not None
    popped = nc._tile_sem_poison_stack.pop()
    assert popped is self._sem_poison
    nc.clear_and_free_semaphores(list(self.sems.allocated().values()))
    nc.all_engine_barrier()


tile.TileContext._lower_ordered_insts = _lower_ordered_insts
tile.TileContext._drain_and_barrier = _drain_and_barrier


# ---------------------------------------------------------------------------
def _bf16(x):
    return np.asarray(x, dtype=np.float32).astype(ml_dtypes.bfloat16)


def _host_prep(inp):
    N = inp["x_nodes"].shape[0]
    batch = np.asarray(inp["batch"]).astype(np.int64)
    ei = np.asarray(inp["edge_index"]).astype(np.int64)
    pos = np.asarray(inp["pos"]).astype(np.float32)
    xn = np.asarray(inp["x_nodes"]).astype(np.float32)
    src, dst = ei[0], ei[1]

    core_of = (batch // IPC).astype(np.int64)
    deg = np.bincount(dst, minlength=N)

    shard_sizes = np.bincount(core_of, minlength=NC)
    NT = int(np.ceil((shard_sizes.max() + 64) / 512.0)) * 4
    NSH = NT * 128
    NTOT = NC * NSH
    ZROW = NTOT

    node_col = np.full(N, -1, np.int64)
    core_nodes = []
    for c in range(NC):
        nodes = np.where(core_of == c)[0]
        order = nodes[np.argsort(deg[nodes], kind="stable")]
        node_col[order] = c * NSH + np.arange(len(order))
        core_nodes.append(order)

    K_t = np.ones(NT, np.int64)
    for c in range(NC):
        d = np.zeros(NSH, np.int64)
        d[: len(core_nodes[c])] = deg[core_nodes[c]]
        K_t = np.maximum(K_t, d.reshape(NT, 128).max(axis=1))
    K_t = K_t.astype(int)
    SLOT_TOT = int(K_t.sum())
    slot0 = np.concatenate([[0], np.cumsum(K_t)])[:-1].astype(int)

    order_e = np.argsort(dst, kind="stable")
    src_sorted = src[order_e]
    row_ptr = np.concatenate([[0], np.cumsum(deg)])
    src_new = node_col[src_sorted]
    edge_idx = np.full((NC, 128, SLOT_TOT), ZROW, np.int32)
    for c in range(NC):
        nodes = core_nodes[c]
        js = np.arange(len(nodes))
        t_of, p_of = js // 128, js % 128
        d_of, r0 = deg[nodes], row_ptr[nodes]
        for t in range(NT):
            sel = np.where(t_of == t)[0]
            if len(sel) == 0:
                continue
            for i in range(K_t[t]):
                has = sel[d_of[sel] > i]
                if len(has) == 0:
                    continue
                edge_idx[c, p_of[has], slot0[t] + i] = src_new[r0[has] + i]

    invdeg = np.ones((NC, 128, NT), np.float32)
    for c in range(NC):
        nodes = core_nodes[c]
        iv = np.ones(NSH, np.float32)
        iv[: len(nodes)] = 1.0 / np.maximum(deg[nodes], 1)
        invdeg[c] = iv.reshape(NT, 128).T

    pidx = np.round(pos / 20.0 * (HW - 1)).astype(np.int64)
    ix, iy = pidx[:, 0], pidx[:, 1]
    gpix_local = (batch - core_of * IPC) * PIX + iy * HW + ix
    pix_own = np.zeros((NC, 128, NT), np.int32)
    for c in range(NC):
        nodes = core_nodes[c]
        pv = np.zeros(NSH, np.int64)
        pv[: len(nodes)] = gpix_local[nodes]
        pix_own[c] = pv.reshape(NT, 128).T.astype(np.int32)

    gx = np.concatenate([xn, pos], axis=1)  # [N, 7]
    gxT = np.zeros((NC, 7, NSH), np.float32)
    gxrows = np.zeros((NC, 128, NT * 8), np.float32)
    for c in range(NC):
        nodes = core_nodes[c]
        g = np.zeros((NSH, 8), np.float32)
        g[: len(nodes), :7] = gx[nodes]
        gxT[c] = g[:, :7].T
        gxrows[c] = g.reshape(NT, 128, 8).transpose(1, 0, 2).reshape(128, NT * 8)

    meta = dict(NT=NT, NSH=NSH, NTOT=NTOT, ZROW=ZROW,
                K_t=[int(k) for k in K_t], slot0=[int(s) for s in slot0],
                SLOT_TOT=SLOT_TOT)
    per_core = dict(edge_idx=edge_idx, invdeg=invdeg, pix_own=pix_own,
                    gxT=gxT, gxrows=gxrows)
    return meta, per_core, node_col


def _cnn_weights(inp):
    w1 = np.asarray(inp["w1"], np.float32).sum(axis=1)      # [16,3,3]
    w2 = np.asarray(inp["w2"], np.float32)
    w3 = np.asarray(inp["w3"], np.float32)
    w4 = np.asarray(inp["w4"], np.float32)
    b1 = np.asarray(inp["b1"], np.float32)
    b2 = np.asarray(inp["b2"], np.float32)
    b3 = np.asarray(inp["b3"], np.float32)
    b4 = np.asarray(inp["b4"], np.float32)

    W1T = np.zeros((9, 8, 128), np.float32)
    W2T = np.zeros((9, 128, 128), np.float32)
    W3T = np.zeros((9, 128, 64), np.float32)
    W4T = np.zeros((9, 128, 8), np.float32)
    for t in range(9):
        dy, dx = t // 3, t % 3
        for i in range(8):
            W1T[t, i, i * 16:(i + 1) * 16] = w1[:, dy, dx]
            W4T[t, i * 16:(i + 1) * 16, i] = w4[0, :, dy, dx]
        for i in range(4):
            W2T[t, i * 16:(i + 1) * 16, i * 32:(i + 1) * 32] = w2[:, :, dy, dx].T
            W2T[t, 64 + i * 16:64 + (i + 1) * 16,
                i * 32:(i + 1) * 32] = w2[:, :, dy, dx].T
            W3T[t, i * 32:(i + 1) * 32, i * 16:(i + 1) * 16] = w3[:, :, dy, dx].T
    B1 = np.tile(b1, 8)[:, None]
    B2 = np.tile(b2, 4)[:, None]
    B3 = np.tile(b3, 4)[:, None]
    B4 = np.repeat(b4, 8)[:, None]
    return dict(W1T=W1T, W2T=W2T, W3T=W3T, W4T=W4T, B1=B1, B2=B2, B3=B3, B4=B4)


def _build(meta, b3_val):
    NT, NSH, NTOT = meta["NT"], meta["NSH"], meta["NTOT"]
    K_t, slot0, SLOT_TOT = meta["K_t"], meta["slot0"], meta["SLOT_TOT"]

    nc = bass.Bass("TRN2", target_bir_lowering=False, debug=False,
                   num_devices=NC)

    ximg = nc.dram_tensor("ximg", [IPC, PIX], F32, kind="ExternalInput")
    gxT_in = nc.dram_tensor("gxT", [7, NSH], BF16, kind="ExternalInput")
    gxrows_in = nc.dram_tensor("gxrows", [128, NT * 8], BF16,
                               kind="ExternalInput")
    eidx_in = nc.dram_tensor("eidx", [128, SLOT_TOT], I32, kind="ExternalInput")
    pix_in = nc.dram_tensor("pix", [128, NT], I32, kind="ExternalInput")
    ivd_in = nc.dram_tensor("ivd", [128, NT], F32, kind="ExternalInput")
    cwd = {}
    for nm, sh in [("W1T", (9, 8, 128)), ("W2T", (9, 128, 128)),
                   ("W3T", (9, 128, 64)), ("W4T", (9, 128, 8))]:
        cwd[nm] = nc.dram_tensor(nm, list(sh), BF16, kind="ExternalInput")
    for nm, p in [("B1", 128), ("B2", 128), ("B3", 64), ("B4", 8)]:
        cwd[nm] = nc.dram_tensor(nm, [p, 1], F32, kind="ExternalInput")
    gwd = {}
    for l in range(4):
        for pre in ("wl", "wr"):
            gwd[f"{pre}{l}"] = nc.dram_tensor(
                f"{pre}{l}", [DIMS[l], DIMS[l + 1]], F32, kind="ExternalInput")
        if l < 3:
            gwd[f"bs{l}"] = nc.dram_tensor(f"bs{l}", [DIMS[l + 1], 1], F32,
                                           kind="ExternalInput")

    out_x1 = nc.dram_tensor("x1", [IPC, PIX], F32, kind="ExternalOutput")
    out_g = nc.dram_tensor("gout", [1, NSH], F32, kind="ExternalOutput")

    with tile.TileContext(nc) as tc:
        with (
            tc.tile_pool(name="big", bufs=1) as big,
            tc.tile_pool(name="const", bufs=1) as cpool,
            tc.tile_pool(name="wide", bufs=3) as widep,
            tc.tile_pool(name="acc", bufs=4) as accp,
            tc.tile_pool(name="stg", bufs=2) as stg,
            tc.tile_pool(name="stg1", bufs=1) as stg1,
            tc.tile_pool(name="psA", bufs=2, space="PSUM") as psA,
            tc.tile_pool(name="psB", bufs=2, space="PSUM") as psB,
            tc.tile_pool(name="psC", bufs=2, space="PSUM") as psC,
            tc.tile_pool(name="dram", bufs=1, space="DRAM") as dram,
        ):
            # ------- constants
            eidx = cpool.tile([128, SLOT_TOT], I32)
            nc.sync.dma_start(eidx[:], eidx_in[:])
            pixt = cpool.tile([128, NT], I32)
            nc.sync.dma_start(pixt[:], pix_in[:])
            ivd = cpool.tile([128, NT], F32)
            nc.sync.dma_start(ivd[:], ivd_in[:])
            ident = cpool.tile([128, 128], BF16)
            make_identity(nc, ident)
            ident32 = cpool.tile([128, 128], F32)
            make_identity(nc, ident32)
            cw = {}
            for nm, (p, f) in [("W1T", (8, 128)), ("W2T", (128, 128)),
                               ("W3T", (128, 64)), ("W4T", (128, 8))]:
                cw[nm] = []
                for t in range(9):
                    w = cpool.tile([p, f], BF16, tag=f"{nm}{t}")
                    nc.sync.dma_start(w[:], cwd[nm][t])
                    cw[nm].append(w)
            for nm in ["B1", "B2", "B3", "B4"]:
                b = cpool.tile(list(cwd[nm].shape), F32, tag=nm)
                nc.sync.dma_start(b[:], cwd[nm][:])
                cw[nm] = b
            gw = {}
            for l in range(4):
                for pre in ("wl", "wr"):
                    t_ = cpool.tile([DIMS[l], DIMS[l + 1]], F32,
                                    tag=f"{pre}{l}")
                    nc.sync.dma_start(t_[:], gwd[f"{pre}{l}"][:])
                    gw[f"{pre}{l}"] = t_
                if l < 3:
                    t_ = cpool.tile([DIMS[l + 1], 1], F32, tag=f"bs{l}")
                    nc.sync.dma_start(t_[:], gwd[f"bs{l}"][:])
                    gw[f"bs{l}"] = t_

            # ------- DRAM scratch
            h0full = dram.tile([NTOT + 1, 8], BF16)
            xfull = [None]
            for l in range(1, 4):
                xf = dram.tile([NTOT + 1, 128], BF16, tag=f"xfull{l}")
                xfull.append(xf)
            ag_h0 = dram.tile([NSH, 8], BF16)
            ag_x = dram.tile([NSH, 128], BF16)
            x1flat = dram.tile([IPC * PIX], F32)

            zrow = cpool.tile([1, 128], BF16)
            nc.any.memset(zrow[:], 0.0)
            nc.sync.dma_start(h0full[NTOT:NTOT + 1, :], zrow[0:1, 0:8])
            for l in range(1, 4):
                nc.sync.dma_start(xfull[l][NTOT:NTOT + 1, :], zrow[0:1, 0:128])

            # ------- CNN
            l0buf = big.tile([8, PPIX], BF16, tag="T1")
            nc.any.memset(l0buf[:], 0.0)
            nc.gpsimd.dma_start(
                l0buf[:].rearrange("p (a b) -> p a b", a=PW)[:, 1:129, 1:129],
                ximg[:].rearrange("p (a b) -> p a b", a=HW))
            l1out = big.tile([128, PPIX], BF16, tag="T2")
            nc.any.memset(l1out[:], 0.0)

            def conv(srcbuf, wts, bt, outbuf, K, M, relu, out_part0=0,
                     src_part0=0, out_padded=True):
                for cg in range(8):
                    for ch in range(4):
                        r0 = cg * 16 + ch * 4
                        ps = psA.tile([M, 512], F32, tag="cnn")
                        for t in range(9):
                            dy, dx = t // 3, t % 3
                            v = srcbuf[src_part0:src_part0 + K, :].rearrange(
                                "p (a b) -> p a b", a=PW)
                            rhs = v[:, r0 + dy:r0 + dy + 4, dx:dx + 128]
                            nc.tensor.matmul(ps[:], wts[t][:], rhs,
                                             start=(t == 0), stop=(t == 8))
                        if out_padded:
                            dst = outbuf[out_part0:out_part0 + M, :].rearrange(
                                "p (a b) -> p a b", a=PW)[:, r0 + 1:r0 + 5,
                                                          1:129]
                        else:
                            dst = outbuf[out_part0:out_part0 + M, :].rearrange(
                                "p (a b) -> p a b", a=HW)[:, r0:r0 + 4, :]
                        if relu:
                            nc.scalar.activation(
                                dst, ps[:].rearrange("p (a b) -> p a b", a=4),
                                mybir.ActivationFunctionType.Relu, bias=bt[:])
                        else:
                            nc.scalar.activation(
                                dst, ps[:].rearrange("p (a b) -> p a b", a=4),
                                mybir.ActivationFunctionType.Copy)

            conv(l0buf, cw["W1T"], cw["B1"], l1out, 8, 128, True)
            l2out = big.tile([128, PPIX], BF16, tag="T3")
            nc.any.memset(l2out[:], 0.0)
            l3out = big.tile([128, PPIX], BF16, tag="T4")
            nc.any.memset(l3out[:], 0.0)
            for g in range(2):
                conv(l1out, [w[g * 64:(g + 1) * 64, :] for w in cw["W2T"]],
                     cw["B2"], l2out, 64, 128, True, src_part0=g * 64)
                if g == 0:
                    conv(l2out, cw["W3T"], cw["B3"], l3out, 128, 64, True)
                else:
                    tmp64 = big.tile([64, PPIX], BF16, tag="T2")
                    nc.any.memset(tmp64[:], 0.0)
                    conv(l2out, cw["W3T"], cw["B3"], tmp64, 128, 64, True)
                    nc.sync.dma_start(l3out[64:128, :], tmp64[0:64, :])
            x1sb = big.tile([8, PIX], BF16, tag="T1")
            conv(l3out, cw["W4T"], cw["B4"], x1sb, 128, 8, False,
                 out_padded=False)

            nc.gpsimd.dma_start(out_x1[:], x1sb[:])
            nc.gpsimd.dma_start(
                x1flat[:].rearrange("(p f) -> p f", p=IPC), x1sb[:])

            # ------- h0 table
            cn = cpool.tile([128, NT], F32)
            x1v = x1flat[:].rearrange("(a b) -> a b", b=1)
            for t in range(NT):
                nc.gpsimd.indirect_dma_start(
                    out=cn[:, t:t + 1], out_offset=None, in_=x1v,
                    in_offset=bass.IndirectOffsetOnAxis(ap=pixt[:, t:t + 1],
                                                        axis=0))
            h0rows = stg1.tile([128, NT * 8], BF16, tag="h0r")
            nc.sync.dma_start(h0rows[:], gxrows_in[:])
            nc.vector.tensor_copy(
                h0rows[:].rearrange("p (a b) -> p a b", b=8)[:, :, 7:8],
                cn[:].rearrange("p (a b) -> p a b", b=1))
            nc.sync.dma_start(
                ag_h0[:].rearrange("(t p) c -> p t c", p=128),
                h0rows[:].rearrange("p (t c) -> p t c", c=8))
            nc.gpsimd.collective_compute(
                "AllGather", mybir.AluOpType.bypass,
                ins=[ag_h0.opt()], outs=[h0full[0:NTOT, :]],
                replica_groups=[list(range(NC))])
            h0T = cpool.tile([8, NSH], BF16)
            nc.sync.dma_start(h0T[0:7, :], gxT_in[:])
            cn_dram = dram.tile([NSH], BF16, tag="cn_dram")
            nc.gpsimd.dma_start(
                cn_dram[:].rearrange("(t p) -> p t", p=128), cn[:])
            nc.sync.dma_start(h0T[7:8, :], cn_dram[:].rearrange("(q f) -> q f", q=1))

            # ------- GNN layers
            xrows = big.tile([128, NSH], BF16, tag="T3")
            xT = h0T
            for l in range(4):
                din, dout = DIMS[l], DIMS[l + 1]
                table = h0full if l == 0 else xfull[l]
                if l < 3:
                    xnewT = big.tile([128, NSH], BF16,
                                     tag=("T1" if l % 2 == 0 else "T2"))
                for cc in range(NSH // 512):
                    mroll = stg.tile([128, 512], F32, tag="mroll")
                    for tt in range(4):
                        t = cc * 4 + tt
                        acc = accp.tile([128, din], F32, tag="acc")
                        kt = K_t[t]
                        ngr = (kt + KG - 1) // KG
                        for g in range(ngr):
                            kg = min(KG, kt - g * KG)
                            wide = widep.tile([128, KG * din], BF16,
                                              tag="wide")
                            for i in range(kg):
                                s = slot0[t] + g * KG + i
                                nc.gpsimd.indirect_dma_start(
                                    out=wide[:, i * din:(i + 1) * din],
                                    out_offset=None, in_=table[:],
                                    in_offset=bass.IndirectOffsetOnAxis(
                                        ap=eidx[:, s:s + 1], axis=0))
                            red = acc if g == 0 else accp.tile(
                                [128, din], F32, tag="tmp")
                            nc.vector.reduce_sum(
                                red[:],
                                wide[:, 0:kg * din].rearrange(
                                    "p (k d) -> p d k", k=kg),
                                axis=mybir.AxisListType.X)
                            if g > 0:
                                nc.vector.tensor_add(acc[:], acc[:], red[:])
                        mrow = accp.tile([128, din], F32, tag="mrow")
                        nc.scalar.activation(
                            mrow[:], acc[:],
                            mybir.ActivationFunctionType.Copy,
                            scale=ivd[:, t:t + 1])
                        p1 = psC.tile([128, 128], F32, tag="tr32")
                        nc.tensor.transpose(p1[0:din, :], mrow[:], ident32[:])
                        nc.scalar.activation(
                            mroll[0:din, tt * 128:(tt + 1) * 128],
                            p1[0:din, :], mybir.ActivationFunctionType.Copy)
                    xroll = stg.tile([128, 512], F32, tag="xroll")
                    nc.vector.tensor_copy(
                        xroll[0:din, :], xT[0:din, cc * 512:(cc + 1) * 512])
                    ps = psB.tile([dout, 512], F32, tag="dense")
                    nc.tensor.matmul(ps[:], gw[f"wl{l}"][:],
                                     mroll[0:din, :], start=True, stop=False)
                    nc.tensor.matmul(ps[:], gw[f"wr{l}"][:],
                                     xroll[0:din, :], start=False, stop=True)
                    if l < 3:
                        nc.scalar.activation(
                            xnewT[0:dout, cc * 512:(cc + 1) * 512], ps[:],
                            mybir.ActivationFunctionType.Relu,
                            bias=gw[f"bs{l}"][:])
                    else:
                        gst = stg.tile([1, 512], F32, tag="gst")
                        nc.scalar.activation(
                            gst[:], ps[:],
                            mybir.ActivationFunctionType.Copy, bias=b3_val)
                        nc.sync.dma_start(out_g[0:1, cc * 512:(cc + 1) * 512],
                                          gst[:])
                if l < 3:
                    for t in range(NT):
                        p2 = psC.tile([128, 128], BF16, tag="tr")
                        nc.tensor.transpose(
                            p2[:], xnewT[:, t * 128:(t + 1) * 128], ident[:])
                        nc.scalar.activation(
                            xrows[:, t * 128:(t + 1) * 128], p2[:],
                            mybir.ActivationFunctionType.Copy)
                    nc.sync.dma_start(
                        ag_x[:].rearrange("(t p) c -> p t c", p=128),
                        xrows[:].rearrange("p (t c) -> p t c", c=128))
                    nc.gpsimd.collective_compute(
                        "AllGather", mybir.AluOpType.bypass,
                        ins=[ag_x.opt()], outs=[xfull[l + 1][0:NTOT, :]],
                        replica_groups=[list(range(NC))])
                    xT = xnewT
    return nc


def kernel(**inputs):
    inp = {k: np.asarray(v) for k, v in inputs.items()}
    meta, per_core, node_col = _host_prep(inp)
    cwn = _cnn_weights(inp)
    b3_val = float(np.asarray(inp["bs3"], np.float32).ravel()[0])

    nc = _build(meta, b3_val)

    xdata = np.asarray(inp["xdata128"], np.float32)
    in_maps = []
    for c in range(NC):
        m = dict(
            ximg=np.ascontiguousarray(
                xdata[c * IPC:(c + 1) * IPC, 0].reshape(IPC, PIX)),
            gxT=_bf16(per_core["gxT"][c]),
            gxrows=_bf16(per_core["gxrows"][c]),
            eidx=np.ascontiguousarray(per_core["edge_idx"][c]),
            pix=np.ascontiguousarray(per_core["pix_own"][c]),
            ivd=np.ascontiguousarray(per_core["invdeg"][c]),
            W1T=_bf16(cwn["W1T"]), W2T=_bf16(cwn["W2T"]),
            W3T=_bf16(cwn["W3T"]), W4T=_bf16(cwn["W4T"]),
            B1=cwn["B1"].astype(np.float32), B2=cwn["B2"].astype(np.float32),
            B3=cwn["B3"].astype(np.float32), B4=cwn["B4"].astype(np.float32),
        )
        for l in range(4):
            m[f"wl{l}"] = np.asarray(inp[f"wl{l}"], np.float32)
            m[f"wr{l}"] = np.asarray(inp[f"wr{l}"], np.float32)
            if l < 3:
                m[f"bs{l}"] = np.asarray(inp[f"bs{l}"], np.float32)[:, None]
        in_maps.append(m)

    from concourse.bass_utils import run_bass_kernel_spmd
    res = run_bass_kernel_spmd(nc, in_maps, core_ids=list(range(NC)))

    N = inp["x_nodes"].shape[0]
    x1 = np.stack([np.asarray(res.results[c]["x1"]).reshape(IPC, 1, HW, HW)
                   for c in range(NC)]).reshape(64, 1, HW, HW)
    gout_new = np.concatenate([np.asarray(res.results[c]["gout"]).ravel()
                               for c in range(NC)])
    gout = gout_new[node_col[np.arange(N)]][:, None].astype(np.float32)
    return x1.astype(np.float32), gout


# revision 13
# speedup vs baseline: 6.1494x; 6.1494x over previous
"""Trainium2 Bass kernel for CNN+GraphSAGE (nn_CNNadgnn_cnn1ch).

Sharding: data-parallel over the 64 images for the CNN (8 imgs/core); GNN
nodes sharded by owning image-group (batch//8 == core), so the CNN->graph
pixel gather is core-local.  Edges partitioned by dst shard; per-layer node
features exchanged with an ncfw AllGather; small weights replicated.

Aggregation: nodes relabeled (per core, sorted by in-degree); for each
128-node tile, slot i gathers the i-th neighbor's row for all 128 nodes via
one indirect DMA; DVE reduce sums slot groups; ACT applies 1/deg as a
per-partition scale; PE transposes produce the feature-major operand for the
dense transforms.  All matmuls in bf16, accumulation in fp32.
"""

import numpy as np
import ml_dtypes

import bass_rust
import concourse.bass as bass
import concourse.mybir as mybir
import concourse.tile as tile
import concourse.tile_utils as tile_utils
from concourse.masks import make_identity
from concourse.vector_clock import ScopedClock

BF16 = mybir.dt.bfloat16
F32 = mybir.dt.float32
I32 = mybir.dt.int32

NC = 8          # cores
IPC = 8         # images per core
HW = 128
PIX = HW * HW   # 16384
PW = HW + 2     # padded row width
PPIX = PW * PW
KG = 16         # gather slots per DVE reduce group
DIMS = [8, 128, 128, 128, 1]

tile_utils.max_sbuf_usage = 206 * 1024

# ---------------------------------------------------------------------------
# This container's walrus supports only 1 sync-wait slot per ISA instruction;
# split extra Tile-assigned waits onto same-engine NOPs placed just before.
MAX_WAITS = 1


def _split_waits(nc, insts):
    out = []
    for inst in insts:
        si = inst.sync_info
        waits = list(si.on_wait) if si is not None and si.on_wait else []
        if len(waits) > MAX_WAITS and not isinstance(
            inst, (tile.TileBranchInst, tile.BassTileLoopBlock)
        ):
            extra, keep = waits[:-MAX_WAITS], waits[-MAX_WAITS:]
            for i in range(0, len(extra), MAX_WAITS):
                nop = bass_rust.InstNoOp(name=f"I-{nc.next_id()}", ins=[], outs=[])
                nop.engine = inst.engine
                nop.sync_info = bass_rust.SyncInfo(
                    on_wait=extra[i:i + MAX_WAITS], on_update=[]
                )
                out.append(nop)
            inst.sync_info = bass_rust.SyncInfo(
                on_wait=keep, on_update=list(si.on_update or [])
            )
        out.append(inst)
    return out


_orig_lower = tile.TileContext._lower_ordered_insts


def _lower_ordered_insts(self, ordered):
    for bb_name in list(ordered.keys()):
        ordered[bb_name] = _split_waits(self.nc, ordered[bb_name])
    return _orig_lower(self, ordered)


def _drain_and_barrier(self, tick_clock, wait_clock):
    nc = self.nc
    drain_inst = nc.sync.drain()
    wait_clock.add_sem_waits(
        drain_inst.ins, ScopedClock({None: tick_clock.global_clock})
    )
    si = drain_inst.ins.sync_info
    waits = list(si.on_wait) if si is not None and si.on_wait else []
    if len(waits) > MAX_WAITS:
        drain_inst.ins.sync_info = bass_rust.SyncInfo(
            on_wait=waits[:MAX_WAITS], on_update=list(si.on_update or [])
        )
        extra = waits[MAX_WAITS:]
        for i in range(0, len(extra), MAX_WAITS):
            nop = nc.sync.nop(nofuse=True)
            nop.ins.sync_info = bass_rust.SyncInfo(
                on_wait=extra[i:i + MAX_WAITS], on_update=[]
            )
    nc.all_engine_barrier()
    assert self.sems is not None
    popped = nc._tile_sem_poison_stack.pop()
    assert popped is self._sem_poison
    nc.clear_and_free_semaphores(list(self.sems.allocated().values()))
    nc.all_engine_barrier()


tile.TileContext._lower_ordered_insts = _lower_ordered_insts
tile.TileContext._drain_and_barrier = _drain_and_barrier


# ---------------------------------------------------------------------------
def _bf16(x):
    return np.asarray(x, dtype=np.float32).astype(ml_dtypes.bfloat16)


def _host_prep(inp):
    N = inp["x_nodes"].shape[0]
    batch = np.asarray(inp["batch"]).astype(np.int64)
    ei = np.asarray(inp["edge_index"]).astype(np.int64)
    pos = np.asarray(inp["pos"]).astype(np.float32)
    xn = np.asarray(inp["x_nodes"]).astype(np.float32)
    src, dst = ei[0], ei[1]

    core_of = (batch // IPC).astype(np.int64)
    deg = np.bincount(dst, minlength=N)

    shard_sizes = np.bincount(core_of, minlength=NC)
    NT = int(np.ceil((shard_sizes.max() + 192) / 512.0)) * 4
    NSH = NT * 128
    NTOT = NC * NSH
    ZROW = (NT - 1) * 128

    node_col = np.full(N, -1, np.int64)
    core_nodes = []
    for c in range(NC):
        nodes = np.where(core_of == c)[0]
        order = nodes[np.argsort(deg[nodes], kind="stable")]
        node_col[order] = c * NSH + np.arange(len(order))
        core_nodes.append(order)

    K_t = np.ones(NT, np.int64)
    for c in range(NC):
        d = np.zeros(NSH, np.int64)
        d[: len(core_nodes[c])] = deg[core_nodes[c]]
        K_t = np.maximum(K_t, d.reshape(NT, 128).max(axis=1))
    K_t = K_t.astype(int)
    SLOT_TOT = int(K_t.sum())
    slot0 = np.concatenate([[0], np.cumsum(K_t)])[:-1].astype(int)

    order_e = np.argsort(dst, kind="stable")
    src_sorted = src[order_e]
    row_ptr = np.concatenate([[0], np.cumsum(deg)])
    src_new = node_col[src_sorted]
    edge_idx = np.full((NC, 128, SLOT_TOT), ZROW, np.int32)
    for c in range(NC):
        nodes = core_nodes[c]
        js = np.arange(len(nodes))
        t_of, p_of = js // 128, js % 128
        d_of, r0 = deg[nodes], row_ptr[nodes]
        for t in range(NT):
            sel = np.where(t_of == t)[0]
            if len(sel) == 0:
                continue
            for i in range(K_t[t]):
                has = sel[d_of[sel] > i]
                if len(has) == 0:
                    continue
                edge_idx[c, p_of[has], slot0[t] + i] = src_new[r0[has] + i]

    invdeg = np.ones((NC, 128, NT), np.float32)
    for c in range(NC):
        nodes = core_nodes[c]
        iv = np.ones(NSH, np.float32)
        iv[: len(nodes)] = 1.0 / np.maximum(deg[nodes], 1)
        invdeg[c] = iv.reshape(NT, 128).T

    pidx = np.round(pos / 20.0 * (HW - 1)).astype(np.int64)
    ix, iy = pidx[:, 0], pidx[:, 1]
    gpix_local = (batch - core_of * IPC) * PIX + iy * HW + ix
    pix_own = np.zeros((NC, 128, NT), np.int32)
    for c in range(NC):
        nodes = core_nodes[c]
        pv = np.zeros(NSH, np.int64)
        pv[: len(nodes)] = gpix_local[nodes]
        pix_own[c] = pv.reshape(NT, 128).T.astype(np.int32)

    gx = np.concatenate([xn, pos], axis=1)  # [N, 7]
    gxT = np.zeros((NC, 7, NSH), np.float32)
    gxrows = np.zeros((NC, 128, NT * 8), np.float32)
    for c in range(NC):
        nodes = core_nodes[c]
        g = np.zeros((NSH, 8), np.float32)
        g[: len(nodes), :7] = gx[nodes]
        gxT[c] = g[:, :7].T
        gxrows[c] = g.reshape(NT, 128, 8).transpose(1, 0, 2).reshape(128, NT * 8)

    meta = dict(NT=NT, NSH=NSH, NTOT=NTOT, ZROW=ZROW,
                K_t=[int(k) for k in K_t], slot0=[int(s) for s in slot0],
                SLOT_TOT=SLOT_TOT)
    per_core = dict(edge_idx=edge_idx, invdeg=invdeg, pix_own=pix_own,
                    gxT=gxT, gxrows=gxrows)
    return meta, per_core, node_col


def _cnn_weights(inp):
    w1 = np.asarray(inp["w1"], np.float32).sum(axis=1)      # [16,3,3]
    w2 = np.asarray(inp["w2"], np.float32)
    w3 = np.asarray(inp["w3"], np.float32)
    w4 = np.asarray(inp["w4"], np.float32)
    b1 = np.asarray(inp["b1"], np.float32)
    b2 = np.asarray(inp["b2"], np.float32)
    b3 = np.asarray(inp["b3"], np.float32)
    b4 = np.asarray(inp["b4"], np.float32)

    W1T = np.zeros((9, 8, 128), np.float32)
    W2T = np.zeros((9, 128, 128), np.float32)
    W3T = np.zeros((9, 128, 64), np.float32)
    W4T = np.zeros((9, 128, 8), np.float32)
    for t in range(9):
        dy, dx = t // 3, t % 3
        for i in range(8):
            W1T[t, i, i * 16:(i + 1) * 16] = w1[:, dy, dx]
            W4T[t, i * 16:(i + 1) * 16, i] = w4[0, :, dy, dx]
        for i in range(4):
            W2T[t, i * 16:(i + 1) * 16, i * 32:(i + 1) * 32] = w2[:, :, dy, dx].T
            W2T[t, 64 + i * 16:64 + (i + 1) * 16,
                i * 32:(i + 1) * 32] = w2[:, :, dy, dx].T
            W3T[t, i * 32:(i + 1) * 32, i * 16:(i + 1) * 16] = w3[:, :, dy, dx].T
    B1 = np.tile(b1, 8)[:, None]
    B2 = np.tile(b2, 4)[:, None]
    B3 = np.tile(b3, 4)[:, None]
    B4 = np.repeat(b4, 8)[:, None]
    return dict(W1T=W1T, W2T=W2T, W3T=W3T, W4T=W4T, B1=B1, B2=B2, B3=B3, B4=B4)


def _build(meta, b3_val):
    NT, NSH, NTOT = meta["NT"], meta["NSH"], meta["NTOT"]
    K_t, slot0, SLOT_TOT = meta["K_t"], meta["slot0"], meta["SLOT_TOT"]

    nc = bass.Bass("TRN2", target_bir_lowering=False, debug=False,
                   num_devices=NC)

    ximg = nc.dram_tensor("ximg", [IPC, PIX], F32, kind="ExternalInput")
    gxT_in = nc.dram_tensor("gxT", [7, NSH], BF16, kind="ExternalInput")
    gxrows_in = nc.dram_tensor("gxrows", [128, NT * 8], BF16,
                               kind="ExternalInput")
    eidx_in = nc.dram_tensor("eidx", [128, SLOT_TOT], I32, kind="ExternalInput")
    pix_in = nc.dram_tensor("pix", [128, NT], I32, kind="ExternalInput")
    ivd_in = nc.dram_tensor("ivd", [128, NT], F32, kind="ExternalInput")
    cwd = {}
    for nm, sh in [("W1T", (9, 8, 128)), ("W2T", (9, 128, 128)),
                   ("W3T", (9, 128, 64)), ("W4T", (9, 128, 8))]:
        cwd[nm] = nc.dram_tensor(nm, list(sh), BF16, kind="ExternalInput")
    for nm, p in [("B1", 128), ("B2", 128), ("B3", 64), ("B4", 8)]:
        cwd[nm] = nc.dram_tensor(nm, [p, 1], F32, kind="ExternalInput")
    gwd = {}
    for l in range(4):
        for pre in ("wl", "wr"):
            gwd[f"{pre}{l}"] = nc.dram_tensor(
                f"{pre}{l}", [DIMS[l], DIMS[l + 1]], F32, kind="ExternalInput")
        if l < 3:
            gwd[f"bs{l}"] = nc.dram_tensor(f"bs{l}", [DIMS[l + 1], 1], F32,
                                           kind="ExternalInput")

    out_x1 = nc.dram_tensor("x1", [IPC, PIX], F32, kind="ExternalOutput")
    out_g = nc.dram_tensor("gout", [1, NSH], F32, kind="ExternalOutput")

    with tile.TileContext(nc) as tc:
        with (
            tc.tile_pool(name="big", bufs=1) as big,
            tc.tile_pool(name="const", bufs=1) as cpool,
            tc.tile_pool(name="wide", bufs=3) as widep,
            tc.tile_pool(name="acc", bufs=4) as accp,
            tc.tile_pool(name="stg", bufs=2) as stg,
            tc.tile_pool(name="stg1", bufs=1) as stg1,
            tc.tile_pool(name="psA", bufs=2, space="PSUM") as psA,
            tc.tile_pool(name="psB", bufs=2, space="PSUM") as psB,
            tc.tile_pool(name="psC", bufs=2, space="PSUM") as psC,
            tc.tile_pool(name="dram", bufs=1, space="DRAM") as dram,
        ):
            # ------- constants
            eidx = cpool.tile([128, SLOT_TOT], I32)
            nc.sync.dma_start(eidx[:], eidx_in[:])
            pixt = cpool.tile([128, NT], I32)
            nc.sync.dma_start(pixt[:], pix_in[:])
            ivd = cpool.tile([128, NT], F32)
            nc.sync.dma_start(ivd[:], ivd_in[:])
            ident = cpool.tile([128, 128], BF16)
            make_identity(nc, ident)
            ident32 = cpool.tile([128, 128], F32)
            make_identity(nc, ident32)
            cw = {}
            for nm, (p, f) in [("W1T", (8, 128)), ("W2T", (128, 128)),
                               ("W3T", (128, 64)), ("W4T", (128, 8))]:
                cw[nm] = []
                for t in range(9):
                    w = cpool.tile([p, f], BF16, tag=f"{nm}{t}")
                    nc.sync.dma_start(w[:], cwd[nm][t])
                    cw[nm].append(w)
            for nm in ["B1", "B2", "B3", "B4"]:
                b = cpool.tile(list(cwd[nm].shape), F32, tag=nm)
                nc.sync.dma_start(b[:], cwd[nm][:])
                cw[nm] = b
            gw = {}
            for l in range(4):
                for pre in ("wl", "wr"):
                    t_ = cpool.tile([DIMS[l], DIMS[l + 1]], F32,
                                    tag=f"{pre}{l}")
                    nc.sync.dma_start(t_[:], gwd[f"{pre}{l}"][:])
                    gw[f"{pre}{l}"] = t_
                if l < 3:
                    t_ = cpool.tile([DIMS[l + 1], 1], F32, tag=f"bs{l}")
                    nc.sync.dma_start(t_[:], gwd[f"bs{l}"][:])
                    gw[f"bs{l}"] = t_

            # ------- DRAM scratch
            h0full = dram.tile([NTOT, 8], BF16, addr_space="Shared")
            xfull = [None]
            for l in range(1, 4):
                xf = dram.tile([NTOT, 128], BF16, tag=f"xfull{l}", addr_space="Shared")
                xfull.append(xf)
            ag_h0 = dram.tile([NSH, 8], BF16)
            ag_x = dram.tile([NSH, 128], BF16)
            x1flat = dram.tile([IPC * PIX], F32)


            # ------- CNN
            l0buf = big.tile([8, PPIX], BF16, tag="T1")
            nc.any.memset(l0buf[:], 0.0)
            nc.gpsimd.dma_start(
                l0buf[:].rearrange("p (a b) -> p a b", a=PW)[:, 1:129, 1:129],
                ximg[:].rearrange("p (a b) -> p a b", a=HW))
            l1out = big.tile([128, PPIX], BF16, tag="T2")
            nc.any.memset(l1out[:], 0.0)

            def conv(srcbuf, wts, bt, outbuf, K, M, relu, out_part0=0,
                     src_part0=0, out_padded=True):
                for cg in range(8):
                    for ch in range(4):
                        r0 = cg * 16 + ch * 4
                        ps = psA.tile([M, 512], F32, tag="cnn")
                        for t in range(9):
                            dy, dx = t // 3, t % 3
                            v = srcbuf[src_part0:src_part0 + K, :].rearrange(
                                "p (a b) -> p a b", a=PW)
                            rhs = v[:, r0 + dy:r0 + dy + 4, dx:dx + 128]
                            nc.tensor.matmul(ps[:], wts[t][:], rhs,
                                             start=(t == 0), stop=(t == 8))
                        if out_padded:
                            dst = outbuf[out_part0:out_part0 + M, :].rearrange(
                                "p (a b) -> p a b", a=PW)[:, r0 + 1:r0 + 5,
                                                          1:129]
                        else:
                            dst = outbuf[out_part0:out_part0 + M, :].rearrange(
                                "p (a b) -> p a b", a=HW)[:, r0:r0 + 4, :]
                        if relu:
                            nc.scalar.activation(
                                dst, ps[:].rearrange("p (a b) -> p a b", a=4),
                                mybir.ActivationFunctionType.Relu, bias=bt[:])
                        else:
                            nc.scalar.activation(
                                dst, ps[:].rearrange("p (a b) -> p a b", a=4),
                                mybir.ActivationFunctionType.Copy)

            conv(l0buf, cw["W1T"], cw["B1"], l1out, 8, 128, True)
            l2out = big.tile([128, PPIX], BF16, tag="T3")
            nc.any.memset(l2out[:], 0.0)
            l3out = big.tile([128, PPIX], BF16, tag="T4")
            nc.any.memset(l3out[:], 0.0)
            for g in range(2):
                conv(l1out, [w[g * 64:(g + 1) * 64, :] for w in cw["W2T"]],
                     cw["B2"], l2out, 64, 128, True, src_part0=g * 64)
                if g == 0:
                    conv(l2out, cw["W3T"], cw["B3"], l3out, 128, 64, True)
                else:
                    tmp64 = big.tile([64, PPIX], BF16, tag="T2")
                    nc.any.memset(tmp64[:], 0.0)
                    conv(l2out, cw["W3T"], cw["B3"], tmp64, 128, 64, True)
                    nc.sync.dma_start(l3out[64:128, :], tmp64[0:64, :])
            x1sb = big.tile([8, PIX], BF16, tag="T1")
            conv(l3out, cw["W4T"], cw["B4"], x1sb, 128, 8, False,
                 out_padded=False)

            nc.gpsimd.dma_start(out_x1[:], x1sb[:])
            nc.gpsimd.dma_start(
                x1flat[:].rearrange("(p f) -> p f", p=IPC), x1sb[:])

            # ------- h0 table
            cn = cpool.tile([128, NT], F32)
            x1v = x1flat[:].rearrange("(a b) -> a b", b=1)
            for t in range(NT):
                nc.gpsimd.indirect_dma_start(
                    out=cn[:, t:t + 1], out_offset=None, in_=x1v,
                    in_offset=bass.IndirectOffsetOnAxis(ap=pixt[:, t:t + 1],
                                                        axis=0))
            h0rows = stg1.tile([128, NT * 8], BF16, tag="h0r")
            nc.sync.dma_start(h0rows[:], gxrows_in[:])
            nc.vector.tensor_copy(
                h0rows[:].rearrange("p (a b) -> p a b", b=8)[:, :, 7:8],
                cn[:].rearrange("p (a b) -> p a b", b=1))
            nc.any.memset(h0rows[:, (NT - 1) * 8:NT * 8], 0.0)
            nc.sync.dma_start(
                ag_h0[:].rearrange("(t p) c -> p t c", p=128),
                h0rows[:].rearrange("p (t c) -> p t c", c=8))
            nc.gpsimd.collective_compute(
                "AllGather", mybir.AluOpType.bypass,
                ins=[ag_h0.opt()], outs=[h0full.opt()],
                replica_groups=[list(range(NC))])
            h0T = cpool.tile([8, NSH], BF16)
            nc.sync.dma_start(h0T[0:7, :], gxT_in[:])
            cn_dram = dram.tile([NSH], BF16, tag="cn_dram")
            nc.gpsimd.dma_start(
                cn_dram[:].rearrange("(t p) -> p t", p=128), cn[:])
            nc.sync.dma_start(h0T[7:8, :], cn_dram[:].rearrange("(q f) -> q f", q=1))

            # ------- GNN layers
            xrows = big.tile([128, NSH], BF16, tag="T3")
            xT = h0T
            for l in range(4):
                din, dout = DIMS[l], DIMS[l + 1]
                table = h0full if l == 0 else xfull[l]
                if l < 3:
                    xnewT = big.tile([128, NSH], BF16,
                                     tag=("T1" if l % 2 == 0 else "T2"))
                for cc in range(NSH // 512):
                    mroll = stg.tile([128, 512], F32, tag="mroll")
                    for tt in range(4):
                        t = cc * 4 + tt
                        acc = accp.tile([128, din], F32, tag="acc")
                        kt = K_t[t]
                        ngr = (kt + KG - 1) // KG
                        for g in range(ngr):
                            kg = min(KG, kt - g * KG)
                            wide = widep.tile([128, KG * din], BF16,
                                              tag="wide")
                            for i in range(kg):
                                s = slot0[t] + g * KG + i
                                nc.gpsimd.indirect_dma_start(
                                    out=wide[:, i * din:(i + 1) * din],
                                    out_offset=None, in_=table[:],
                                    in_offset=bass.IndirectOffsetOnAxis(
                                        ap=eidx[:, s:s + 1], axis=0))
                            red = acc if g == 0 else accp.tile(
                                [128, din], F32, tag="tmp")
                            nc.vector.reduce_sum(
                                red[:],
                                wide[:, 0:kg * din].rearrange(
                                    "p (k d) -> p d k", k=kg),
                                axis=mybir.AxisListType.X)
                            if g > 0:
                                nc.vector.tensor_add(acc[:], acc[:], red[:])
                        mrow = accp.tile([128, din], F32, tag="mrow")
                        nc.scalar.activation(
                            mrow[:], acc[:],
                            mybir.ActivationFunctionType.Copy,
                            scale=ivd[:, t:t + 1])
                        p1 = psC.tile([128, 128], F32, tag="tr32")
                        nc.tensor.transpose(p1[0:din, :], mrow[:], ident32[:])
                        nc.scalar.activation(
                            mroll[0:din, tt * 128:(tt + 1) * 128],
                            p1[0:din, :], mybir.ActivationFunctionType.Copy)
                    xroll = stg.tile([128, 512], F32, tag="xroll")
                    nc.vector.tensor_copy(
                        xroll[0:din, :], xT[0:din, cc * 512:(cc + 1) * 512])
                    ps = psB.tile([dout, 512], F32, tag="dense")
                    nc.tensor.matmul(ps[:], gw[f"wl{l}"][:],
                                     mroll[0:din, :], start=True, stop=False)
                    nc.tensor.matmul(ps[:], gw[f"wr{l}"][:],
                                     xroll[0:din, :], start=False, stop=True)
                    if l < 3:
                        nc.scalar.activation(
                            xnewT[0:dout, cc * 512:(cc + 1) * 512], ps[:],
                            mybir.ActivationFunctionType.Relu,
                            bias=gw[f"bs{l}"][:])
                    else:
                        gst = stg.tile([1, 512], F32, tag="gst")
                        nc.scalar.activation(
                            gst[:], ps[:],
                            mybir.ActivationFunctionType.Copy, bias=b3_val)
                        nc.sync.dma_start(out_g[0:1, cc * 512:(cc + 1) * 512],
                                          gst[:])
                if l < 3:
                    nc.any.memset(xnewT[:, (NT - 1) * 128:NSH], 0.0)
                    for t in range(NT):
                        p2 = psC.tile([128, 128], BF16, tag="tr")
                        nc.tensor.transpose(
                            p2[:], xnewT[:, t * 128:(t + 1) * 128], ident[:])
                        nc.scalar.activation(
                            xrows[:, t * 128:(t + 1) * 128], p2[:],
                            mybir.ActivationFunctionType.Copy)
                    nc.sync.dma_start(
                        ag_x[:].rearrange("(t p) c -> p t c", p=128),
                        xrows[:].rearrange("p (t c) -> p t c", c=128))
                    nc.gpsimd.collective_compute(
                        "AllGather", mybir.AluOpType.bypass,
                        ins=[ag_x.opt()], outs=[xfull[l + 1].opt()],
                        replica_groups=[list(range(NC))])
                    xT = xnewT
    return nc


def kernel(**inputs):
    inp = {k: np.asarray(v) for k, v in inputs.items()}
    meta, per_core, node_col = _host_prep(inp)
    cwn = _cnn_weights(inp)
    b3_val = float(np.asarray(inp["bs3"], np.float32).ravel()[0])

    nc = _build(meta, b3_val)

    xdata = np.asarray(inp["xdata128"], np.float32)
    in_maps = []
    for c in range(NC):
        m = dict(
            ximg=np.ascontiguousarray(
                xdata[c * IPC:(c + 1) * IPC, 0].reshape(IPC, PIX)),
            gxT=_bf16(per_core["gxT"][c]),
            gxrows=_bf16(per_core["gxrows"][c]),
            eidx=np.ascontiguousarray(per_core["edge_idx"][c]),
            pix=np.ascontiguousarray(per_core["pix_own"][c]),
            ivd=np.ascontiguousarray(per_core["invdeg"][c]),
            W1T=_bf16(cwn["W1T"]), W2T=_bf16(cwn["W2T"]),
            W3T=_bf16(cwn["W3T"]), W4T=_bf16(cwn["W4T"]),
            B1=cwn["B1"].astype(np.float32), B2=cwn["B2"].astype(np.float32),
            B3=cwn["B3"].astype(np.float32), B4=cwn["B4"].astype(np.float32),
        )
        for l in range(4):
            m[f"wl{l}"] = np.asarray(inp[f"wl{l}"], np.float32)
            m[f"wr{l}"] = np.asarray(inp[f"wr{l}"], np.float32)
            if l < 3:
                m[f"bs{l}"] = np.asarray(inp[f"bs{l}"], np.float32)[:, None]
        in_maps.append(m)

    from concourse.bass_utils import run_bass_kernel_spmd
    res = run_bass_kernel_spmd(nc, in_maps, core_ids=list(range(NC)))
    import os, time
    nrep = int(os.environ.get("KERNEL_TIME_RUNS", "0"))
    if nrep:
        ts = []
        for _ in range(nrep):
            t0 = time.time()
            run_bass_kernel_spmd(nc, in_maps, core_ids=list(range(NC)))
            ts.append(time.time() - t0)
        print(f"exec wall min/median over {nrep}: "
              f"{min(ts):.3f}s / {sorted(ts)[len(ts)//2]:.3f}s")

    N = inp["x_nodes"].shape[0]
    x1 = np.stack([np.asarray(res.results[c]["x1"]).reshape(IPC, 1, HW, HW)
                   for c in range(NC)]).reshape(64, 1, HW, HW)
    gout_new = np.concatenate([np.asarray(res.results[c]["gout"]).ravel()
                               for c in range(NC)])
    gout = gout_new[node_col[np.arange(N)]][:, None].astype(np.float32)
    return x1.astype(np.float32), gout
